# revision 16
# baseline (speedup 1.0000x reference)
"""Causal attention (B=4, S=2048, D=1024) on 8 trn2 NeuronCores.

Sharding: core c = (batch b = c//2, query-group h = c%2). Each core computes
K and V projections for its WHOLE batch (duplicated with its pair core — no
collectives, no DRAM staging round-trips) plus Q for its own 8 interleaved
query tiles of 128 rows. Tiles are interleaved (t % 4 in {0,3} for h=0,
{1,2} for h=1) so both cores of a pair have the same causal work profile and
the SPMD program is structurally identical on every core.

All matmul operands are bf16 (fp32 PSUM accumulation): halves DMA bytes and
SBUF footprint vs f32r, so x / K / V / Q all stay SBUF-resident — the only
HBM traffic is inputs in (~13 MB) and the output out (4 MB) per core.

Device kernel per core:
  KT[o,k] = sum_d WkT[d,o] xT[d,k]          k = 0..2047   (full batch)
  V[s,o]  = sum_d xT[d,s]  WvT[d,o]         s = 0..2047
  QT[o,q] = sum_d WqT[d,o] xTq[d,q]         q = core's 1024 rows
                                            (Wq pre-scaled by 1/32 on host)
  per sorted q-tile position j (L = (2j+2)*128 keys, both h fit under L):
    S[q,k] = sum_o QT[o,q] KT[o,k];  last 256 cols += mask (covers diag
             block + the 128-col overhang the other h-core doesn't need)
    P = exp(S)  (no rowmax subtraction: |S| <= ~6, exp is fp32-safe;
             masked cols are -1e30 -> exp underflows to exactly 0)
    rowsum fused via activation accum_out
    C[q,:] = sum_k P^T[k,q] V[k,:]  (P^T via PE transpose, bf16)
    out = C * (1/rowsum)
"""

import os
import sys
from contextlib import ExitStack

import ml_dtypes
import numpy as np

sys.path.insert(0, "/opt/trn_rl_repo")

import concourse.bass as bass
import concourse.tile as tile
from concourse import bacc, mybir
from concourse.bass_utils import run_bass_kernel_spmd

F32 = mybir.dt.float32
BF16 = mybir.dt.bfloat16
NPBF16 = ml_dtypes.bfloat16
P = 128
B, S, D = 4, 2048, 1024
NDC = D // P                     # 8 contraction chunks of 128
NQT = 8                          # q-tiles of 128 rows per core
QCORE = NQT * P                  # 1024 q rows per core
TILES = {
    0: [t for t in range(16) if t % 4 in (0, 3)],
    1: [t for t in range(16) if t % 4 in (1, 2)],
}
# position j covers L_j = (2j+2)*128 key columns: the max over the two
# h-cores' causal needs at that sorted position; the mask input zeroes the
# per-core overhang (at most 128 cols, always inside the last 256).
LJS = [(2 * j + 2) * P for j in range(NQT)]

_COMPILED = {}
LAST_RESULTS = None


def _score_chunks(L):
    """Split L key cols into matmul chunks <=512; last chunk is the 256-wide
    mask window."""
    pre = L - 256
    chunks = []
    off = 0
    while pre - off >= 512:
        chunks.append((off, 512, False))
        off += 512
    if pre - off:
        chunks.append((off, pre - off, False))
    chunks.append((pre, 256, True))
    return chunks


def _emit_body(nc, tc, rctx, aps, version=1):
    if version == 1:
        xT, xTq, wqT, wkT, wvT, masks, ident, out, pspool = aps
        cc = None
    else:
        (xT, xTq, wqT, wkT, wvT, masks, ident, out,
         ktag_in, ktag_out, vag_in, vag_out, pspool) = aps
        cc = [[0, 1], [2, 3], [4, 5], [6, 7]]
    KH = S if version == 1 else S // 2   # key/value rows projected locally
    copy_ctr = [0]

    def copy_out(dst, src):
        # alternate PSUM->SBUF copies between vector and scalar engines
        copy_ctr[0] += 1
        if copy_ctr[0] % 2:
            nc.vector.tensor_copy(dst, src)
        else:
            nc.scalar.copy(dst, src)

    cpool = rctx.enter_context(tc.tile_pool(name="const", bufs=1))
    identsb = cpool.tile([P, P], BF16)
    masksb = cpool.tile([P, NQT, 256], F32)
    ktpool = rctx.enter_context(tc.tile_pool(name="ktp", bufs=1))
    kt_sb = ktpool.tile([P, NDC, S], BF16)     # KT: [o%128, o//128, k]
    vpool = rctx.enter_context(tc.tile_pool(name="vp", bufs=1))
    v_sb = vpool.tile([P, S // P, D], BF16)    # V: [s%128, s//128, o]
    qtpool = rctx.enter_context(tc.tile_pool(name="qtp", bufs=1))
    qt_sb = qtpool.tile([P, NDC, QCORE], BF16)  # QT: [o%128, o//128, q]

    with tc.tile_pool(name="wts", bufs=1) as wpool:
        wk_sb = wpool.tile([P, NDC, D], BF16)
        wv_sb = wpool.tile([P, NDC, D], BF16)
        wq_sb = wpool.tile([P, NDC, D], BF16)
        xt_sb = wpool.tile([P, NDC, KH], BF16)  # xT: [d%128, d//128, s]
        xtq_sb = wpool.tile([P, NDC, QCORE], BF16)
        if version == 2:
            kstg = wpool.tile([P, NDC, KH], BF16)     # staged KT [p, c, k]
            vstg = wpool.tile([P, KH // P, D], BF16)  # staged V [p, s//128, o]

        # ---- input DMAs, batched (DIRECT2D issue cost is ~0.6us each,
        # serialized on the issuing sequencer — so few big transfers) -------
        def ld(dst, w, cols):
            # one dma per o/s-half: [128, 8, |cols|] <- w[(d p), cols]
            nc.sync.dma_start(
                dst[:, :, cols], w[:, cols].rearrange("(d p) o -> p d o", p=P)
            )

        ld(wk_sb, wkT, slice(0, 256))
        ld(xt_sb, xT, slice(0, 512))
        ld(wk_sb, wkT, slice(256, 512))
        ld(wk_sb, wkT, slice(512, 1024))
        for g in range(1, KH // 512):
            ld(xt_sb, xT, slice(g * 512, (g + 1) * 512))
        ld(wv_sb, wvT, slice(0, 512))
        ld(wv_sb, wvT, slice(512, 1024))
        ld(wq_sb, wqT, slice(0, 512))
        ld(wq_sb, wqT, slice(512, 1024))
        nc.sync.dma_start(
            xtq_sb[:], xTq.rearrange("(d p) q -> p d q", p=P)
        )
        nc.sync.dma_start(identsb[:], ident[:])
        nc.sync.dma_start(masksb[:], masks[:])

        # ---- K projection: KT[o, own keys] -------------------------------
        kdst = kt_sb if version == 1 else kstg
        for ks in range(KH // 512):
            for c in range(NDC):
                ps = pspool.tile([P, 512], F32, tag="mm", bufs=3)
                for d in range(NDC):
                    nc.tensor.matmul(
                        ps[:],
                        wk_sb[:, d, c * P : (c + 1) * P],
                        xt_sb[:, d, ks * 512 : (ks + 1) * 512],
                        start=(d == 0),
                        stop=(d == NDC - 1),
                    )
                copy_out(kdst[:, c, ks * 512 : (ks + 1) * 512], ps[:])
            if version == 2:
                # stage this k-half to DRAM for the pairwise gather
                nc.sync.dma_start(
                    ktag_in[:, ks * 512 : (ks + 1) * 512].rearrange(
                        "(c p) k -> p c k", p=P
                    ),
                    kstg[:, :, ks * 512 : (ks + 1) * 512],
                )
        if version == 2:
            nc.gpsimd.collective_compute(
                "AllGather", mybir.AluOpType.bypass, replica_groups=cc,
                ins=[ktag_in[:]], outs=[ktag_out[:]],
            )

        # ---- V projection: V[own rows, o] --------------------------------
        vdst = v_sb if version == 1 else vstg
        for st_i in range(KH // P):
            for oh in range(2):
                ps = pspool.tile([P, 512], F32, tag="mm", bufs=3)
                for d in range(NDC):
                    nc.tensor.matmul(
                        ps[:],
                        xt_sb[:, d, st_i * P : (st_i + 1) * P],
                        wv_sb[:, d, oh * 512 : (oh + 1) * 512],
                        start=(d == 0),
                        stop=(d == NDC - 1),
                    )
                copy_out(vdst[:, st_i, oh * 512 : (oh + 1) * 512], ps[:])
            if version == 2 and st_i == 3:
                # first V half staged -> gather it while the second half
                # computes; kt readback rides the gpsimd queue in between
                # (K gather already done, so it doesn't block the V gather).
                # Gather A rows = [s 0:512 | s 1024:1536] (rank-major).
                nc.sync.dma_start(
                    vag_in[0:512, :].rearrange("(t p) o -> p t o", p=P),
                    vstg[:, 0:4, :],
                )
                nc.gpsimd.collective_compute(
                    "AllGather", mybir.AluOpType.bypass, replica_groups=cc,
                    ins=[vag_in[0:512, :]], outs=[vag_out[0:1024, :]],
                )
                # kt readback on the sync queue: waits only on the K-gather
                # semaphore (done during V proj), not behind gather A on the
                # serial gpsimd queue
                for r in range(2):
                    nc.sync.dma_start(
                        kt_sb[:, :, r * KH : (r + 1) * KH],
                        ktag_out[r * D : (r + 1) * D, :].rearrange(
                            "(c p) k -> p c k", p=P
                        ),
                    )
                # vagA available: v s-tiles 0-3 and 8-11
                nc.gpsimd.dma_start(
                    v_sb[:, 0:4, :],
                    vag_out[0:512, :].rearrange("(t p) o -> p t o", p=P),
                )
                nc.gpsimd.dma_start(
                    v_sb[:, 8:12, :],
                    vag_out[512:1024, :].rearrange("(t p) o -> p t o", p=P),
                )
        if version == 2:
            # Gather B rows = [s 512:1024 | s 1536:2048]
            nc.sync.dma_start(
                vag_in[512:1024, :].rearrange("(t p) o -> p t o", p=P),
                vstg[:, 4:8, :],
            )
            nc.gpsimd.collective_compute(
                "AllGather", mybir.AluOpType.bypass, replica_groups=cc,
                ins=[vag_in[512:1024, :]], outs=[vag_out[1024:2048, :]],
            )

        # ---- Q projection: QT[o, q] --------------------------------------
        for qs in range(QCORE // 512):
            for c in range(NDC):
                ps = pspool.tile([P, 512], F32, tag="mm", bufs=3)
                for d in range(NDC):
                    nc.tensor.matmul(
                        ps[:],
                        wq_sb[:, d, c * P : (c + 1) * P],
                        xtq_sb[:, d, qs * 512 : (qs + 1) * 512],
                        start=(d == 0),
                        stop=(d == NDC - 1),
                    )
                copy_out(qt_sb[:, c, qs * 512 : (qs + 1) * 512], ps[:])

        if version == 2:
            # vagB readback: v s-tiles 4-7 and 12-15
            nc.gpsimd.dma_start(
                v_sb[:, 4:8, :],
                vag_out[1024:1536, :].rearrange("(t p) o -> p t o", p=P),
            )
            nc.gpsimd.dma_start(
                v_sb[:, 12:16, :],
                vag_out[1536:2048, :].rearrange("(t p) o -> p t o", p=P),
            )

    # ---- attention, software-pipelined per q-tile position ---------------
    with tc.tile_pool(name="sp", bufs=2) as sp, tc.tile_pool(
        name="pp", bufs=2
    ) as pp, tc.tile_pool(name="stats", bufs=4) as stp, tc.tile_pool(
        name="atp", bufs=4
    ) as atp, tc.tile_pool(name="cp", bufs=2) as cp:
        state = {}

        def emit_scores(j):
            L = LJS[j]
            ssb = sp.tile([P, L], F32, tag="ssb", name=f"ssb{j}")
            for off, w, is_mask in _score_chunks(L):
                ps = pspool.tile([P, 512], F32, tag="mm", bufs=3)
                for c in range(NDC):
                    nc.tensor.matmul(
                        ps[:, :w],
                        qt_sb[:, c, j * P : (j + 1) * P],
                        kt_sb[:, c, off : off + w],
                        start=(c == 0),
                        stop=(c == NDC - 1),
                    )
                if is_mask:
                    nc.vector.tensor_add(
                        ssb[:, off : off + w], ps[:, :w], masksb[:, j, :]
                    )
                else:
                    copy_out(ssb[:, off : off + w], ps[:, :w])
            state[j] = ssb

        def emit_softmax_pv(j):
            L = LJS[j]
            ssb = state.pop(j)
            psb = pp.tile([P, L], BF16, tag="psb", name=f"psb{j}")
            sumv = stp.tile([P, 1], F32, tag="sumv", name=f"sumv{j}")
            nc.scalar.activation(
                psb[:],
                ssb[:],
                mybir.ActivationFunctionType.Exp,
                scale=1.0,
                accum_out=sumv[:],
            )
            rcp = stp.tile([P, 1], F32, tag="rcp", name=f"rcp{j}")
            nc.vector.reciprocal(rcp[:], sumv[:])

            co0 = pspool.tile([P, 512], F32, tag="co", bufs=2, name=f"co0_{j}")
            co1 = pspool.tile([P, 512], F32, tag="co", bufs=2, name=f"co1_{j}")
            nkt = L // P
            for k in range(nkt):
                tp = pspool.tile([P, P], BF16, tag="tp", bufs=3, name=f"tp{j}_{k}")
                nc.tensor.transpose(tp[:], psb[:, k * P : (k + 1) * P], identsb[:])
                at = atp.tile([P, P], BF16, tag="at", name=f"at{j}_{k}")
                copy_out(at[:], tp[:])
                nc.tensor.matmul(
                    co0[:], at[:], v_sb[:, k, 0:512],
                    start=(k == 0), stop=(k == nkt - 1),
                )
                nc.tensor.matmul(
                    co1[:], at[:], v_sb[:, k, 512:1024],
                    start=(k == 0), stop=(k == nkt - 1),
                )
            csb = cp.tile([P, D], F32, tag="csb", name=f"csb{j}")
            nc.vector.tensor_scalar_mul(csb[:, 0:512], co0[:], rcp[:])
            nc.scalar.activation(
                csb[:, 512:1024],
                co1[:],
                mybir.ActivationFunctionType.Copy,
                scale=rcp[:],
            )
            nc.sync.dma_start(out[j * P : (j + 1) * P, :], csb[:])

        emit_scores(0)
        for j in range(1, NQT):
            emit_scores(j)
            emit_softmax_pv(j - 1)
        emit_softmax_pv(NQT - 1)


def _build(version=1):
    nc = bacc.Bacc("TRN2", target_bir_lowering=False, debug=False, num_devices=8)

    kh = S if version == 1 else S // 2
    xT = nc.dram_tensor("xT", [D, kh], BF16, kind="ExternalInput").ap()
    xTq = nc.dram_tensor("xTq", [D, QCORE], BF16, kind="ExternalInput").ap()
    wqT = nc.dram_tensor("wqT", [D, D], BF16, kind="ExternalInput").ap()
    wkT = nc.dram_tensor("wkT", [D, D], BF16, kind="ExternalInput").ap()
    wvT = nc.dram_tensor("wvT", [D, D], BF16, kind="ExternalInput").ap()
    masks = nc.dram_tensor("masks", [P, NQT, 256], F32, kind="ExternalInput").ap()
    ident = nc.dram_tensor("ident", [P, P], BF16, kind="ExternalInput").ap()
    out = nc.dram_tensor("out", [QCORE, D], F32, kind="ExternalOutput").ap()
    extra = ()
    if version == 2:
        ktag_in = nc.dram_tensor("ktag_in", [D, S // 2], BF16).ap()
        ktag_out = nc.dram_tensor("ktag_out", [2 * D, S // 2], BF16).ap()
        vag_in = nc.dram_tensor("vag_in", [S // 2, D], BF16).ap()
        vag_out = nc.dram_tensor("vag_out", [S, D], BF16).ap()
        extra = (ktag_in, ktag_out, vag_in, vag_out)

    with tile.TileContext(nc) as tc, ExitStack() as rctx:
        pspool = rctx.enter_context(
            tc.tile_pool(name="ps", bufs=2, space=bass.MemorySpace.PSUM)
        )
        aps = (xT, xTq, wqT, wkT, wvT, masks, ident, out) + extra + (pspool,)
        _emit_body(nc, tc, rctx, aps, version=version)

    nc.compile()
    return nc


def _prep_inputs(x, Wk, Wq, Wv, version=1):
    x = np.asarray(x, dtype=np.float32)
    wqT = np.ascontiguousarray(
        (np.asarray(Wq, np.float32).T / 32.0).astype(NPBF16)
    )
    wkT = np.ascontiguousarray(np.asarray(Wk, np.float32).T.astype(NPBF16))
    wvT = np.ascontiguousarray(np.asarray(Wv, np.float32).T.astype(NPBF16))
    ident = np.eye(P, dtype=NPBF16)

    mask_by_h = {}
    for h in (0, 1):
        mk = np.empty((P, NQT, 256), np.float32)
        for j, t in enumerate(TILES[h]):
            base = LJS[j] - 256
            col = base + np.arange(256)[None, :]
            row = t * P + np.arange(P)[:, None]
            mk[:, j, :] = np.where(col <= row, 0.0, -1e30)
        mask_by_h[h] = mk

    in_maps = []
    for c in range(8):
        b, h = c // 2, c % 2
        xTb = np.ascontiguousarray(x[b].T.astype(NPBF16))
        qcols = np.concatenate([np.arange(t * P, (t + 1) * P) for t in TILES[h]])
        xt_in = xTb if version == 1 else np.ascontiguousarray(
            xTb[:, h * (S // 2) : (h + 1) * (S // 2)]
        )
        in_maps.append(
            {
                "xT": xt_in,
                "xTq": np.ascontiguousarray(xTb[:, qcols]),
                "wqT": wqT,
                "wkT": wkT,
                "wvT": wvT,
                "masks": mask_by_h[h],
                "ident": ident,
            }
        )
    return in_maps


VERSION = int(os.environ.get("BASS_KERNEL_VERSION", "1"))


def kernel(x, Wk, Wq, Wv):
    global LAST_RESULTS
    if VERSION not in _COMPILED:
        _COMPILED[VERSION] = _build(VERSION)
    nc = _COMPILED[VERSION]
    in_maps = _prep_inputs(x, Wk, Wq, Wv, version=VERSION)
    trace = bool(int(os.environ.get("BASS_KERNEL_TRACE", "0")))
    res = run_bass_kernel_spmd(nc, in_maps, list(range(8)), trace=trace)
    LAST_RESULTS = res
    out = np.empty((B, S, D), np.float32)
    for c in range(8):
        b, h = c // 2, c % 2
        oc = res.results[c]["out"]
        for j, t in enumerate(TILES[h]):
            out[b, t * P : (t + 1) * P, :] = oc[j * P : (j + 1) * P, :]
    return out
